# revision 11
# baseline (speedup 1.0000x reference)
"""LookAheadMask kernel for Trainium2 — in-place, merged diag writes, v4.

out[b, r, c] = 1.0 if c > r else x[b, r, c], for x of shape (8, 4096, 4096) f32.

Sharding: batch dim across 8 NeuronCores (data parallel, no communication).
The output aliases the input buffer (lowering_input_output_aliases={0: 0}),
so the strictly-lower triangle never moves: ~4 MiB read + ~34 MiB written
per core.

Measured head model (v1-v3 traces): SP ring ~8.3 ns/descriptor and ~440+
GB/s on big descriptors; ACT ring ~440 GB/s for >=4 KiB descriptors but
~19 ns/desc below ~2 KiB; framework preamble ~9 us.  Plan:

  - Diag gather (4096 x 1 KiB descs, desc-bound, unavoidable) runs on SP,
    in 4 chunks so the 4 affine_selects pipeline behind it. Nothing else
    is queued on SP until the selects fire: v4 showed byte-DMAs queued
    behind the gather delay its completion semaphore by ~30 us.
  - No scatter: diag blocks leave SBUF as the leading 128 cols of
    [128 x 1024] rectangles sourced from diag_sel, a [128, 32*1024] tile
    pre-memset to 1.0 in quarters (2 on DVE, 2 on gpsimd) into which
    gpsimd affine_selects only the diagonal 128-col windows.
  - ACT streams the bulk pure-ones rectangles and both merged chunks
    (4 KiB descs run at ~430 GB/s on ACT); after the selects, SP does the
    clipped blocks 24-31 plus the five widest pure rects. The SWDGE queue
    is not used: its drain latency measured ~50+ us regardless of size.
"""

import numpy as np

S = 4096
P = 128
NB = S // P  # 32
N_CORES = 8
W = 256  # diag gather window width (1 KiB descriptors)
MW = 1024  # merged diag-rectangle width (4 KiB descriptors)
DB = P * S + P  # element stride between consecutive diagonal blocks

SP_BLOCKS = [0, 1, 2, 3, 4]  # pure-ones rects on the SP ring (post-gather)
ACT_A = list(range(5, 17))  # ACT pure rects issued before the merged chunks
ACT_B = list(range(17, 24))  # ACT pure rects issued after

_cached = None


def _build():
    from concourse import bass, mybir

    nc = bass.Bass(target_bir_lowering=True, enable_partition_id=False)
    x = nc.dram_tensor("x", [S, S], mybir.dt.float32, kind="ExternalInput")
    out = nc.dram_tensor("out", [S, S], mybir.dt.float32, kind="ExternalOutput")

    N_WRITES = 24 + 2 + 8  # pure ones + merged chunks + clipped blocks

    def pure_ones(eng, blocks, ones, sem):
        for i in blocks:
            r0 = i * P
            w = S - r0 - MW
            eng.dma_start(
                out=out[r0 : r0 + P, r0 + MW : S], in_=ones[:, :w]
            ).then_inc(sem, 16)

    def gather_chunk(eng, b0, nblk, gsa):
        eng.dma_start(
            out=bass.AP(diag_in2_h[0], b0 * W + W, [[NB * W, P], [W, nblk], [1, W]]),
            in_=bass.AP(
                x, (b0 + 1) * DB + P - W, [[S, P], [DB, nblk], [1, W]]
            ),
        ).then_inc(gsa, 16)

    diag_in2_h = [None]

    with (
        nc.Block() as block,
        nc.semaphore("dsem") as dsem,  # all output-write DMA completions
        nc.semaphore("gsa") as gsa,  # gather chunks (SP ring)
        nc.semaphore("msem") as msem,  # ones memset done
        nc.semaphore("m2") as m2,  # diag_sel DVE-quarter memsets done
        nc.semaphore("asem") as asem,  # affine_selects done
        nc.sbuf_tensor("ones", [P, S - MW], mybir.dt.float32) as ones,
        nc.sbuf_tensor("diag_in2", [P, NB * W], mybir.dt.float32) as diag_in2,
        nc.sbuf_tensor("diag_sel", [P, NB * MW], mybir.dt.float32) as diag_sel,
    ):
        diag_in2_h[0] = diag_in2

        @block.vector
        def _(vector: bass.BassVectorEngine):
            vector.memset(ones[:, :], 1.0).then_inc(msem, 1)
            vector.memset(diag_sel[:, : 8 * MW], 1.0).then_inc(m2, 1)
            vector.memset(diag_sel[:, 8 * MW : 16 * MW], 1.0).then_inc(m2, 1)

        @block.sync
        def _(sync: bass.BassEngine):
            # Diag gather on the fast-descriptor SP head, 4 chunks of 8
            # blocks. Block 0's window would start before the tensor, so it
            # gets its own 128-col load (chunk 0 covers blocks 1-7).
            sync.dma_start(
                out=bass.AP(diag_in2, W - P, [[NB * W, P], [1, P]]),
                in_=x[0:P, 0:P],
            ).then_inc(gsa, 16)
            gather_chunk(sync, 0, 7, gsa)
            gather_chunk(sync, 7, 8, gsa)
            gather_chunk(sync, 15, 8, gsa)
            gather_chunk(sync, 23, 8, gsa)
            # Keep SP's queue empty behind the gather: byte work queued
            # here before asem fires delays the gather-completion sems.
            sync.wait_ge(asem, 4)
            # Blocks 24-31: clipped merged rect covers the whole row span.
            for b in range(24, 32):
                r0 = b * P
                w = S - r0
                sync.dma_start(
                    out=out[r0 : r0 + P, r0:S],
                    in_=bass.AP(diag_sel, b * MW, [[NB * MW, P], [1, w]]),
                ).then_inc(dsem, 16)
            pure_ones(sync, SP_BLOCKS, ones, dsem)
            sync.wait_ge(dsem, 16 * N_WRITES)

        @block.scalar
        def _(scalar: bass.BassEngine):
            scalar.wait_ge(msem, 1)
            pure_ones(scalar, ACT_A, ones, dsem)
            scalar.wait_ge(asem, 2)
            # Merged rectangles for diag blocks 0-15 (4 KiB descriptors —
            # ~430 GB/s on ACT; only <=2 KiB descriptors are slow there).
            scalar.dma_start(
                out=bass.AP(out, 0, [[S, P], [DB, 16], [1, MW]]),
                in_=bass.AP(diag_sel, 0, [[NB * MW, P], [MW, 16], [1, MW]]),
            ).then_inc(dsem, 16)
            scalar.wait_ge(asem, 3)
            # Merged rectangles for diag blocks 16-23.
            scalar.dma_start(
                out=bass.AP(out, 16 * DB, [[S, P], [DB, 8], [1, MW]]),
                in_=bass.AP(diag_sel, 16 * MW, [[NB * MW, P], [MW, 8], [1, MW]]),
            ).then_inc(dsem, 16)
            pure_ones(scalar, ACT_B, ones, dsem)

        @block.gpsimd
        def _(gpsimd: bass.BassGpSimd):
            gpsimd.memset(diag_sel[:, 16 * MW : 24 * MW], 1.0)
            gpsimd.memset(diag_sel[:, 24 * MW :], 1.0)

            def select(q):
                # iota[p, c] = p - c; keep x where p >= c (at/below diag),
                # else 1.0. Writes ONLY the 128 diag cols of each 1024-wide
                # window; the other 896 cols keep the memset 1.0.
                gpsimd.affine_select(
                    out=bass.AP(
                        diag_sel, q * 8 * MW, [[NB * MW, P], [MW, 8], [1, P]]
                    ),
                    in_=bass.AP(
                        diag_in2,
                        q * 8 * W + W - P,
                        [[NB * W, P], [W, 8], [1, P]],
                    ),
                    pattern=[[0, 8], [-1, P]],
                    base=0,
                    channel_multiplier=1,
                    compare_op=mybir.AluOpType.is_ge,
                    fill=1.0,
                ).then_inc(asem, 1)

            gpsimd.wait_ge(gsa, 32)  # block 0 + chunk 1-7
            gpsimd.wait_ge(m2, 1)
            select(0)
            gpsimd.wait_ge(gsa, 48)
            gpsimd.wait_ge(m2, 2)
            select(1)
            gpsimd.wait_ge(gsa, 64)
            select(2)
            gpsimd.wait_ge(gsa, 80)
            select(3)

    nc.finalize()
    return nc


def _make_runner():
    """Compile-once runner: jit(shard_map(_body)) over 8 cores with the
    output aliased to the (donated) input — mirrors
    bass2jax.run_bass_via_pjrt, plus lowering_input_output_aliases."""
    global _cached
    if _cached is not None:
        return _cached

    import jax
    from jax.sharding import Mesh, PartitionSpec
    from jax.experimental.shard_map import shard_map
    from concourse import bass2jax

    bass2jax.install_neuronx_cc_hook()
    nc = _build()

    def _body(xg):
        outs = bass2jax._bass_exec_p.bind(
            xg,
            out_avals=(jax.core.ShapedArray((S, S), np.float32),),
            in_names=("x",),
            out_names=("out",),
            lowering_input_output_aliases=((0, 0),),
            sim_require_finite=True,
            sim_require_nnan=True,
            nc=nc,
        )
        return tuple(outs)

    devices = jax.devices()[:N_CORES]
    assert len(devices) == N_CORES, f"need {N_CORES} devices, have {len(devices)}"
    mesh = Mesh(np.asarray(devices), ("core",))
    sharded = jax.jit(
        shard_map(
            _body,
            mesh=mesh,
            in_specs=(PartitionSpec("core"),),
            out_specs=(PartitionSpec("core"),),
            check_rep=False,
        ),
        donate_argnums=(0,),
        keep_unused=True,
    )
    _cached = (nc, sharded)
    return _cached


class _Result:
    def __init__(self, exec_time_ns=None, mean_exec_time_ns=None):
        self.exec_time_ns = exec_time_ns
        self.mean_exec_time_ns = mean_exec_time_ns


def _run(x_full: np.ndarray, trace: bool = False):
    nc, sharded = _make_runner()
    x_full = np.asarray(x_full, dtype=np.float32)
    xg = np.ascontiguousarray(x_full.reshape(N_CORES * S, S))

    if not trace:
        out = sharded(xg)[0]
        return np.asarray(out).reshape(N_CORES, S, S), _Result()

    # Trace path (test.py only): NTFF profile around the execution, then the
    # same gauge/perfetto pipeline run_bass_kernel_spmd uses under axon.
    import glob
    import os
    import tempfile

    from antenv.axon_hooks import get_axon_ntff_profile_hook
    from concourse import bass_utils as BU

    neff_dir = tempfile.mkdtemp()
    hook = get_axon_ntff_profile_hook()
    with hook(neff_dir, [0]):
        out = np.asarray(sharded(xg)[0])

    ntffs = glob.glob(os.path.join(neff_dir, "*_body*.ntff"))
    if not ntffs:
        return out.reshape(N_CORES, S, S), _Result()

    sharepath = BU.upload_artifacts(neff_dir)
    profile = BU.gauge.profiler.Profile(
        profile_path=BU.FishPath(neff_dir),
        kernel_dev_mode=True,
        profile_on_exit=False,
        bass_kernel=nc.m,
        offline_processing=True,
        fname="*_body*",
        annotate_hlo=False,
        metadata={"artifacts_path": sharepath},
    )
    perf = BU._process_ntff_profile(
        profile,
        neff_dir,
        nc,
        list(range(N_CORES)),
        None,
        False,
        {},
        trace_events=False,
    )
    return out.reshape(N_CORES, S, S), _Result(
        perf.exec_time_ns, perf.mean_exec_time_ns
    )


def kernel(x: np.ndarray) -> np.ndarray:
    out, _ = _run(x, trace=False)
    return out


# revision 12
# speedup vs baseline: 1.0029x; 1.0029x over previous
"""LookAheadMask kernel for Trainium2 — in-place, merged diag writes, v4.

out[b, r, c] = 1.0 if c > r else x[b, r, c], for x of shape (8, 4096, 4096) f32.

Sharding: batch dim across 8 NeuronCores (data parallel, no communication).
The output aliases the input buffer (lowering_input_output_aliases={0: 0}),
so the strictly-lower triangle never moves: ~4 MiB read + ~34 MiB written
per core.

Measured head model (v1-v3 traces): SP ring ~8.3 ns/descriptor and ~440+
GB/s on big descriptors; ACT ring ~440 GB/s for >=4 KiB descriptors but
~19 ns/desc below ~2 KiB; framework preamble ~9 us.  Plan:

  - Diag gather (4096 x 1 KiB descs, desc-bound, unavoidable) is split
    2+2 chunks across BOTH rings. Chunks <= ~2.2k descriptors complete in
    ~12 us even with byte work queued behind (v2); larger single-ring
    gathers degrade to ~16 ns/desc once concurrent byte traffic shares
    the 16 SDMA engines (v4/v5), stalling the selects until ~60 us.
  - No scatter: diag blocks leave SBUF as the leading 128 cols of
    [128 x 1024] rectangles sourced from diag_sel, a [128, 32*1024] tile
    pre-memset to 1.0 in quarters (2 on DVE, 2 on gpsimd) into which
    gpsimd affine_selects only the diagonal 128-col windows.
  - ACT streams wide pure rects then merged c0 (gated only on SP-side
    selects, never on ACT's own gather sems). SP streams the remaining
    pure rects, then merged c1 and the clipped blocks 24-31 (narrow
    descriptors are ~2.3x cheaper on SP than ACT). SWDGE is unused
    (drain latency ~50+ us regardless of size).
"""

import numpy as np

S = 4096
P = 128
NB = S // P  # 32
N_CORES = 8
W = 256  # diag gather window width (1 KiB descriptors)
MW = 1024  # merged diag-rectangle width (4 KiB descriptors)
DB = P * S + P  # element stride between consecutive diagonal blocks

SP_BLOCKS = [0, 1, 2, 3, 4, 17, 18, 19, 20, 21, 22, 23]  # pure rects on SP
ACT_BLOCKS = list(range(5, 17))  # wide pure rects on ACT

_cached = None


def _build():
    from concourse import bass, mybir

    nc = bass.Bass(target_bir_lowering=True, enable_partition_id=False)
    x = nc.dram_tensor("x", [S, S], mybir.dt.float32, kind="ExternalInput")
    out = nc.dram_tensor("out", [S, S], mybir.dt.float32, kind="ExternalOutput")

    N_WRITES = 24 + 2 + 8  # pure ones + merged chunks + clipped blocks

    def pure_ones(eng, blocks, ones, sem):
        for i in blocks:
            r0 = i * P
            w = S - r0 - MW
            eng.dma_start(
                out=out[r0 : r0 + P, r0 + MW : S], in_=ones[:, :w]
            ).then_inc(sem, 16)

    def gather_chunk(eng, b0, nblk, gsa):
        eng.dma_start(
            out=bass.AP(diag_in2_h[0], b0 * W + W, [[NB * W, P], [W, nblk], [1, W]]),
            in_=bass.AP(
                x, (b0 + 1) * DB + P - W, [[S, P], [DB, nblk], [1, W]]
            ),
        ).then_inc(gsa, 16)

    diag_in2_h = [None]

    with (
        nc.Block() as block,
        nc.semaphore("dsem") as dsem,  # all output-write DMA completions
        nc.semaphore("gsa") as gsa,  # gather chunks on the SP ring
        nc.semaphore("gsb") as gsb,  # gather chunks on the ACT ring
        nc.semaphore("msem") as msem,  # ones memset done
        nc.semaphore("m2") as m2,  # diag_sel DVE-quarter memsets done
        nc.semaphore("asem") as asem,  # affine_selects done
        nc.sbuf_tensor("ones", [P, S - MW], mybir.dt.float32) as ones,
        nc.sbuf_tensor("diag_in2", [P, NB * W], mybir.dt.float32) as diag_in2,
        nc.sbuf_tensor("diag_sel", [P, NB * MW], mybir.dt.float32) as diag_sel,
    ):
        diag_in2_h[0] = diag_in2

        @block.vector
        def _(vector: bass.BassVectorEngine):
            vector.memset(ones[:, :], 1.0).then_inc(msem, 1)
            vector.memset(diag_sel[:, : 8 * MW], 1.0).then_inc(m2, 1)
            vector.memset(diag_sel[:, 8 * MW : 16 * MW], 1.0).then_inc(m2, 1)

        @block.sync
        def _(sync: bass.BassEngine):
            # Diag gather on the fast-descriptor SP head, 4 chunks of 8
            # blocks. Block 0's window would start before the tensor, so it
            # gets its own 128-col load (chunk 0 covers blocks 1-7).
            sync.dma_start(
                out=bass.AP(diag_in2, W - P, [[NB * W, P], [1, P]]),
                in_=x[0:P, 0:P],
            ).then_inc(gsa, 16)
            gather_chunk(sync, 0, 7, gsa)
            gather_chunk(sync, 7, 8, gsa)
            sync.wait_ge(msem, 1)
            pure_ones(sync, SP_BLOCKS, ones, dsem)
            sync.wait_ge(asem, 3)
            # Merged rectangles for diag blocks 16-23.
            sync.dma_start(
                out=bass.AP(out, 16 * DB, [[S, P], [DB, 8], [1, MW]]),
                in_=bass.AP(diag_sel, 16 * MW, [[NB * MW, P], [MW, 8], [1, MW]]),
            ).then_inc(dsem, 16)
            sync.wait_ge(asem, 4)
            # Blocks 24-31: clipped merged rect covers the whole row span.
            for b in range(24, 32):
                r0 = b * P
                w = S - r0
                sync.dma_start(
                    out=out[r0 : r0 + P, r0:S],
                    in_=bass.AP(diag_sel, b * MW, [[NB * MW, P], [1, w]]),
                ).then_inc(dsem, 16)
            sync.wait_ge(dsem, 16 * N_WRITES)

        @block.scalar
        def _(scalar: bass.BassEngine):
            gather_chunk(scalar, 15, 8, gsb)
            gather_chunk(scalar, 23, 8, gsb)
            scalar.wait_ge(msem, 1)
            pure_ones(scalar, ACT_BLOCKS, ones, dsem)
            scalar.wait_ge(asem, 2)
            # Merged rectangles for diag blocks 0-15 (4 KiB descriptors —
            # ~430 GB/s on ACT; only <=2 KiB descriptors are slow there).
            # Gated only on SP-side selects, never on ACT's own gathers.
            scalar.dma_start(
                out=bass.AP(out, 0, [[S, P], [DB, 16], [1, MW]]),
                in_=bass.AP(diag_sel, 0, [[NB * MW, P], [MW, 16], [1, MW]]),
            ).then_inc(dsem, 16)

        @block.gpsimd
        def _(gpsimd: bass.BassGpSimd):
            gpsimd.memset(diag_sel[:, 16 * MW : 24 * MW], 1.0)
            gpsimd.memset(diag_sel[:, 24 * MW :], 1.0)

            def select(q):
                # iota[p, c] = p - c; keep x where p >= c (at/below diag),
                # else 1.0. Writes ONLY the 128 diag cols of each 1024-wide
                # window; the other 896 cols keep the memset 1.0.
                gpsimd.affine_select(
                    out=bass.AP(
                        diag_sel, q * 8 * MW, [[NB * MW, P], [MW, 8], [1, P]]
                    ),
                    in_=bass.AP(
                        diag_in2,
                        q * 8 * W + W - P,
                        [[NB * W, P], [W, 8], [1, P]],
                    ),
                    pattern=[[0, 8], [-1, P]],
                    base=0,
                    channel_multiplier=1,
                    compare_op=mybir.AluOpType.is_ge,
                    fill=1.0,
                ).then_inc(asem, 1)

            gpsimd.wait_ge(gsa, 32)  # block 0 + blocks 1-7
            gpsimd.wait_ge(m2, 1)
            select(0)
            gpsimd.wait_ge(gsa, 48)  # blocks 8-15
            gpsimd.wait_ge(m2, 2)
            select(1)
            gpsimd.wait_ge(gsb, 16)  # blocks 16-23
            select(2)
            gpsimd.wait_ge(gsb, 32)  # blocks 24-31
            select(3)

    nc.finalize()
    return nc


def _make_runner():
    """Compile-once runner: jit(shard_map(_body)) over 8 cores with the
    output aliased to the (donated) input — mirrors
    bass2jax.run_bass_via_pjrt, plus lowering_input_output_aliases."""
    global _cached
    if _cached is not None:
        return _cached

    import jax
    from jax.sharding import Mesh, PartitionSpec
    from jax.experimental.shard_map import shard_map
    from concourse import bass2jax

    bass2jax.install_neuronx_cc_hook()
    nc = _build()

    def _body(xg):
        outs = bass2jax._bass_exec_p.bind(
            xg,
            out_avals=(jax.core.ShapedArray((S, S), np.float32),),
            in_names=("x",),
            out_names=("out",),
            lowering_input_output_aliases=((0, 0),),
            sim_require_finite=True,
            sim_require_nnan=True,
            nc=nc,
        )
        return tuple(outs)

    devices = jax.devices()[:N_CORES]
    assert len(devices) == N_CORES, f"need {N_CORES} devices, have {len(devices)}"
    mesh = Mesh(np.asarray(devices), ("core",))
    sharded = jax.jit(
        shard_map(
            _body,
            mesh=mesh,
            in_specs=(PartitionSpec("core"),),
            out_specs=(PartitionSpec("core"),),
            check_rep=False,
        ),
        donate_argnums=(0,),
        keep_unused=True,
    )
    _cached = (nc, sharded)
    return _cached


class _Result:
    def __init__(self, exec_time_ns=None, mean_exec_time_ns=None):
        self.exec_time_ns = exec_time_ns
        self.mean_exec_time_ns = mean_exec_time_ns


def _run(x_full: np.ndarray, trace: bool = False):
    nc, sharded = _make_runner()
    x_full = np.asarray(x_full, dtype=np.float32)
    xg = np.ascontiguousarray(x_full.reshape(N_CORES * S, S))

    if not trace:
        out = sharded(xg)[0]
        return np.asarray(out).reshape(N_CORES, S, S), _Result()

    # Trace path (test.py only): NTFF profile around the execution, then the
    # same gauge/perfetto pipeline run_bass_kernel_spmd uses under axon.
    import glob
    import os
    import tempfile

    from antenv.axon_hooks import get_axon_ntff_profile_hook
    from concourse import bass_utils as BU

    neff_dir = tempfile.mkdtemp()
    hook = get_axon_ntff_profile_hook()
    with hook(neff_dir, [0]):
        out = np.asarray(sharded(xg)[0])

    ntffs = glob.glob(os.path.join(neff_dir, "*_body*.ntff"))
    if not ntffs:
        return out.reshape(N_CORES, S, S), _Result()

    sharepath = BU.upload_artifacts(neff_dir)
    profile = BU.gauge.profiler.Profile(
        profile_path=BU.FishPath(neff_dir),
        kernel_dev_mode=True,
        profile_on_exit=False,
        bass_kernel=nc.m,
        offline_processing=True,
        fname="*_body*",
        annotate_hlo=False,
        metadata={"artifacts_path": sharepath},
    )
    perf = BU._process_ntff_profile(
        profile,
        neff_dir,
        nc,
        list(range(N_CORES)),
        None,
        False,
        {},
        trace_events=False,
    )
    return out.reshape(N_CORES, S, S), _Result(
        perf.exec_time_ns, perf.mean_exec_time_ns
    )


def kernel(x: np.ndarray) -> np.ndarray:
    out, _ = _run(x, trace=False)
    return out


# revision 13
# speedup vs baseline: 1.1836x; 1.1801x over previous
"""LookAheadMask kernel for Trainium2 — in-place, merged diag writes.

out[b, r, c] = 1.0 if c > r else x[b, r, c], for x of shape (8, 4096, 4096) f32.

Sharding: batch dim across 8 NeuronCores (data parallel, no communication).

The output aliases the input buffer (lowering_input_output_aliases={0: 0}
through the BIR-lowering/NKI path), so the strictly-lower triangle never
moves. Per-core work is ~4 MiB of HBM reads + ~34 MiB of HBM writes.

Measured DMA-head behavior (v1/v2 traces): big-descriptor writes stream at
~430-470 GB/s per HWDGE ring; small-descriptor DMAs are head-limited at
~6-8 ns/desc on the SP ring but ~23 ns/desc on the ACT ring, and a
512 B-descriptor scatter costs ~25-60 us wherever it runs. So:

  - The 1 KiB-descriptor diag gather (4096 descs, unavoidable: the diag
    band is 4096 scattered 512 B row segments) runs entirely on the SP
    ring, split in two chunks to pipeline the selects.
  - There is NO scatter. A [128, 32*1024] SBUF tile (diag_sel) is
    pre-memset to 1.0 (split across DVE and gpsimd); gpsimd affine_selects
    only the 128-wide diagonal columns of each block into it; each diag
    block then leaves SBUF as the leading 128 cols of a [128 x 1024]
    4 KiB-descriptor rectangle (byte-bound, not desc-bound).
  - Pure-ones rectangles cover cols >= blockstart+1024 from a [128, 3072]
    ones tile; blocks 24-31 are fully covered by the (clipped) merged
    rectangles.
  - Two pure-ones rectangles go through the gpsimd SWDGE queue to measure
    a third DMA head; the rest are balanced SP/ACT.
"""

import numpy as np

S = 4096
P = 128
NB = S // P  # 32
N_CORES = 8
W = 256  # diag gather window width (1 KiB descriptors)
MW = 1024  # merged diag-rectangle width (4 KiB descriptors)
DB = P * S + P  # element stride between consecutive diagonal blocks

SWDGE_BLOCKS = [12, 16]  # pure-ones rects issued on the SWDGE queue
SP_BLOCKS = [0]  # pure-ones rects on the SP ring
ACT_BLOCKS = [i for i in range(24) if i not in SWDGE_BLOCKS + SP_BLOCKS]

_cached = None


def _build():
    from concourse import bass, mybir

    nc = bass.Bass(target_bir_lowering=True, enable_partition_id=False)
    x = nc.dram_tensor("x", [S, S], mybir.dt.float32, kind="ExternalInput")
    out = nc.dram_tensor("out", [S, S], mybir.dt.float32, kind="ExternalOutput")

    N_WRITES = 24 + 2 + 8  # pure ones + merged chunks + clipped blocks

    def pure_ones(eng, blocks, ones, dsem):
        for i in blocks:
            r0 = i * P
            w = S - r0 - MW
            eng.dma_start(
                out=out[r0 : r0 + P, r0 + MW : S], in_=ones[:, :w]
            ).then_inc(dsem, 16)

    with (
        nc.Block() as block,
        nc.semaphore("dsem") as dsem,  # all output-write DMA completions
        nc.semaphore("gsa") as gsa,  # gather chunks (SP ring)
        nc.semaphore("msem") as msem,  # ones memset done
        nc.semaphore("m2") as m2,  # diag_sel DVE-half memset done
        nc.semaphore("asem") as asem,  # affine_selects done
        nc.sbuf_tensor("ones", [P, S - MW], mybir.dt.float32) as ones,
        nc.sbuf_tensor("diag_in2", [P, NB * W], mybir.dt.float32) as diag_in2,
        nc.sbuf_tensor("diag_sel", [P, NB * MW], mybir.dt.float32) as diag_sel,
    ):

        @block.vector
        def _(vector: bass.BassVectorEngine):
            vector.memset(ones[:, :], 1.0).then_inc(msem, 1)
            vector.memset(diag_sel[:, : 16 * MW], 1.0).then_inc(m2, 1)

        @block.sync
        def _(sync: bass.BassEngine):
            # Diag gather, 1 KiB descriptors, all on the fast SP head.
            # Block 0's window would start before the tensor: own 128-col load.
            sync.dma_start(
                out=bass.AP(diag_in2, W - P, [[NB * W, P], [1, P]]),
                in_=x[0:P, 0:P],
            ).then_inc(gsa, 16)
            sync.dma_start(
                out=bass.AP(diag_in2, W, [[NB * W, P], [W, 15], [1, W]]),
                in_=bass.AP(x, DB + P - W, [[S, P], [DB, 15], [1, W]]),
            ).then_inc(gsa, 16)
            sync.dma_start(
                out=bass.AP(diag_in2, 16 * W, [[NB * W, P], [W, 16], [1, W]]),
                in_=bass.AP(x, 16 * DB + P - W, [[S, P], [DB, 16], [1, W]]),
            ).then_inc(gsa, 16)
            sync.wait_ge(msem, 1)
            pure_ones(sync, SP_BLOCKS, ones, dsem)
            # Merged rectangles for diag blocks 0-15: [128 x 1024] each,
            # leading 128 cols are the selected diag, rest ones.
            sync.wait_ge(asem, 1)
            sync.dma_start(
                out=bass.AP(out, 0, [[S, P], [DB, 16], [1, MW]]),
                in_=bass.AP(diag_sel, 0, [[NB * MW, P], [MW, 16], [1, MW]]),
            ).then_inc(dsem, 16)
            sync.wait_ge(asem, 2)
            # Blocks 24-31: merged rect clipped at the right edge covers the
            # whole remaining row span [r0, S). Narrow descriptors cost
            # ~8 ns each on SP vs ~19 ns on ACT, and SP idles here anyway.
            for b in range(24, 32):
                r0 = b * P
                w = S - r0
                sync.dma_start(
                    out=out[r0 : r0 + P, r0:S],
                    in_=bass.AP(diag_sel, b * MW, [[NB * MW, P], [1, w]]),
                ).then_inc(dsem, 16)
            sync.wait_ge(dsem, 16 * N_WRITES)

        @block.scalar
        def _(scalar: bass.BassEngine):
            scalar.wait_ge(msem, 1)
            pure_ones(scalar, ACT_BLOCKS, ones, dsem)
            scalar.wait_ge(asem, 2)
            # Merged rectangles for diag blocks 16-23.
            scalar.dma_start(
                out=bass.AP(out, 16 * DB, [[S, P], [DB, 8], [1, MW]]),
                in_=bass.AP(
                    diag_sel, 16 * MW, [[NB * MW, P], [MW, 8], [1, MW]]
                ),
            ).then_inc(dsem, 16)

        @block.gpsimd
        def _(gpsimd: bass.BassGpSimd):
            gpsimd.memset(diag_sel[:, 16 * MW :], 1.0)
            # iota[p, c] = p - (c % 128); keep x where >= 0 (at/below diag).
            # Select ONLY the 128 diag cols of each 1024-wide window; the
            # other 896 cols stay at the memset 1.0.
            gpsimd.wait_ge(gsa, 32)  # block 0 + blocks 1-15
            gpsimd.wait_ge(m2, 1)
            gpsimd.affine_select(
                out=bass.AP(diag_sel, 0, [[NB * MW, P], [MW, 16], [1, P]]),
                in_=bass.AP(diag_in2, W - P, [[NB * W, P], [W, 16], [1, P]]),
                pattern=[[0, 16], [-1, P]],
                base=0,
                channel_multiplier=1,
                compare_op=mybir.AluOpType.is_ge,
                fill=1.0,
            ).then_inc(asem, 1)
            gpsimd.wait_ge(msem, 1)
            pure_ones(gpsimd, SWDGE_BLOCKS[:1], ones, dsem)
            gpsimd.wait_ge(gsa, 48)  # blocks 16-31
            gpsimd.affine_select(
                out=bass.AP(
                    diag_sel, 16 * MW, [[NB * MW, P], [MW, 16], [1, P]]
                ),
                in_=bass.AP(
                    diag_in2, 16 * W + W - P, [[NB * W, P], [W, 16], [1, P]]
                ),
                pattern=[[0, 16], [-1, P]],
                base=0,
                channel_multiplier=1,
                compare_op=mybir.AluOpType.is_ge,
                fill=1.0,
            ).then_inc(asem, 1)
            pure_ones(gpsimd, SWDGE_BLOCKS[1:], ones, dsem)

    nc.finalize()
    return nc


def _make_runner():
    """Compile-once runner: jit(shard_map(_body)) over 8 cores with the
    output aliased to the (donated) input — mirrors
    bass2jax.run_bass_via_pjrt, plus lowering_input_output_aliases."""
    global _cached
    if _cached is not None:
        return _cached

    import jax
    from jax.sharding import Mesh, PartitionSpec
    from jax.experimental.shard_map import shard_map
    from concourse import bass2jax

    bass2jax.install_neuronx_cc_hook()
    nc = _build()

    def _body(xg):
        outs = bass2jax._bass_exec_p.bind(
            xg,
            out_avals=(jax.core.ShapedArray((S, S), np.float32),),
            in_names=("x",),
            out_names=("out",),
            lowering_input_output_aliases=((0, 0),),
            sim_require_finite=True,
            sim_require_nnan=True,
            nc=nc,
        )
        return tuple(outs)

    devices = jax.devices()[:N_CORES]
    assert len(devices) == N_CORES, f"need {N_CORES} devices, have {len(devices)}"
    mesh = Mesh(np.asarray(devices), ("core",))
    sharded = jax.jit(
        shard_map(
            _body,
            mesh=mesh,
            in_specs=(PartitionSpec("core"),),
            out_specs=(PartitionSpec("core"),),
            check_rep=False,
        ),
        donate_argnums=(0,),
        keep_unused=True,
    )
    _cached = (nc, sharded)
    return _cached


class _Result:
    def __init__(self, exec_time_ns=None, mean_exec_time_ns=None):
        self.exec_time_ns = exec_time_ns
        self.mean_exec_time_ns = mean_exec_time_ns


def _run(x_full: np.ndarray, trace: bool = False):
    nc, sharded = _make_runner()
    x_full = np.asarray(x_full, dtype=np.float32)
    xg = np.ascontiguousarray(x_full.reshape(N_CORES * S, S))

    if not trace:
        out = sharded(xg)[0]
        return np.asarray(out).reshape(N_CORES, S, S), _Result()

    # Trace path (test.py only): NTFF profile around the execution, then the
    # same gauge/perfetto pipeline run_bass_kernel_spmd uses under axon.
    import glob
    import os
    import tempfile

    from antenv.axon_hooks import get_axon_ntff_profile_hook
    from concourse import bass_utils as BU

    neff_dir = tempfile.mkdtemp()
    hook = get_axon_ntff_profile_hook()
    with hook(neff_dir, [0]):
        out = np.asarray(sharded(xg)[0])

    ntffs = glob.glob(os.path.join(neff_dir, "*_body*.ntff"))
    if not ntffs:
        return out.reshape(N_CORES, S, S), _Result()

    sharepath = BU.upload_artifacts(neff_dir)
    profile = BU.gauge.profiler.Profile(
        profile_path=BU.FishPath(neff_dir),
        kernel_dev_mode=True,
        profile_on_exit=False,
        bass_kernel=nc.m,
        offline_processing=True,
        fname="*_body*",
        annotate_hlo=False,
        metadata={"artifacts_path": sharepath},
    )
    perf = BU._process_ntff_profile(
        profile,
        neff_dir,
        nc,
        list(range(N_CORES)),
        None,
        False,
        {},
        trace_events=False,
    )
    return out.reshape(N_CORES, S, S), _Result(
        perf.exec_time_ns, perf.mean_exec_time_ns
    )


def kernel(x: np.ndarray) -> np.ndarray:
    out, _ = _run(x, trace=False)
    return out
